# revision 44
# baseline (speedup 1.0000x reference)
"""Multi-head attention (AttnProcessor2_0) on 8 TRN2 NeuronCores.

Problem: B=2, S=4096, C=640, H=10, Dh=64.
  q/k/v = hs @ W{q,k,v}.T ; per-head scores = q k^T / 8 ; softmax ;
  out = probs v ; y = out @ Wo.T + b_out + hs

Sharding (no collectives): core c -> batch b=c//4, query block g=c%4
(1024 queries).  Each core recomputes full K/V for its batch (head-dim
on partitions), computes its own S/4 x S attention block, output
projection, bias+residual.  Host passes hidden states TRANSPOSED and
ROLLED by the query offset so the same SPMD program works on every
core (softmax+PV are permutation-invariant along the key axis).

Device layout (feature-on-partition, token-on-free):
  kT [640, 4096] (5 chunks of 128 = 2 heads each)
  qT [5][128, 1024] pair layout: head 2i on partitions 0:64, head
     2i+1 on 64:128 (natural projection output, no zero padding)
  QK row-tiled pairs: two K=64 matmuls (row groups 0-1 / 2-3 via
     base-partition slicing) run CONCURRENTLY on the PE -> both heads'
     [128 keys x 512 q] score blocks in ~512 cycles instead of 1024.
  v  [4096, 650] (65-stride per head: 64 cols + ones col -> softmax
     denominators fall out of the PV matmul as PSUM row 64)
  probs: scoresT in PSUM -> ScalarE exp -> bf16 SBUF
  normalization: reciprocal_approx_fast of denom row, rank-1 PE outer
     product to broadcast across partitions, DVE mult into pair-packed
     attn tiles [128, 1024] (head 2i rows 0:64, head 2i+1 rows 64:128)
  oproj: pair-packed -> 5 matmuls of full K=128 contraction per
     (128-out-chunk, 512q), Wo tiles loaded as direct [128,128] DMAs.
Loop order: head-pair OUTER, query-tile inner -> background projection
windows are 2x wider; K/Q/V projections and oproj(it0) ride the PE
slack inside the attention loop (2 spare PSUM banks, tag "pp").
All matmuls bf16 (f32 PSUM accumulation).
"""

import sys

if "/opt/trn_rl_repo" not in sys.path:
    sys.path.insert(0, "/opt/trn_rl_repo")

from contextlib import ExitStack

import ml_dtypes
import numpy as np

import concourse.bass as bass
import concourse.tile as tile
from concourse import mybir
from concourse.bass import ts

BF16 = mybir.dt.bfloat16
F32 = mybir.dt.float32
F8 = mybir.dt.float8e4
DR = mybir.MatmulPerfMode.DoubleRow

B, S, C = 2, 4096, 640
H, DH = 10, 64
NCORES = 8
GROUP = 4  # cores per batch element
SQ = S // GROUP  # 1024 queries per core
SCALE = 0.125  # 1/sqrt(64)
CCH = C // 128  # 5 feature chunks (2 heads each)
NJT = S // 512  # 8 key tiles for K proj
NJC = S // 128  # 32 key chunks for attention
NIT = SQ // 512  # 2 query tiles
VST = DH + 1  # 65: per-head stride in v tiles (ones col appended)
NPAIR = H // 2  # 5 head pairs

# DVE-offloaded exp: fp8e4m3 bits of exp(SCALE*x) are approximately the
# affine x*A8+B8 (Schraudolph bit trick), so a single DVE tensor_scalar
# with int8 output replaces a ScalarE ACTIVATE for ~25% of the softmax --
# the two engines split the 41.9M-element exp stream.  The +-3% sawtooth
# is random across keys and self-normalizes through the denominators.
EXP_A8 = SCALE * 8.0 / np.log(2.0)
EXP_B8 = 55.54


def build_nc() -> bass.Bass:
    nc = bass.Bass()
    # hsT/W{q,k,v} travel as fp8e4 with hs pre-scaled by 1/8 and W by 8 on
    # the host (products exact): both land in fp8's normal range, and the
    # K/Q/V projections run DoubleRow (256-row virtual contraction, ~1.77x).
    hsT = nc.declare_dram_parameter("hsT", [C, S], F8, isOutput=False)
    res = nc.declare_dram_parameter("res", [C, SQ], F32, isOutput=False)
    wqT = nc.declare_dram_parameter("wqT", [C, C], F8, isOutput=False)
    wkT = nc.declare_dram_parameter("wkT", [C, C], F8, isOutput=False)
    wvT = nc.declare_dram_parameter("wvT", [C, C], F8, isOutput=False)
    woT = nc.declare_dram_parameter("woT", [C, C], BF16, isOutput=False)
    out = nc.declare_dram_parameter("out", [C, SQ], F32, isOutput=True)

    with ExitStack() as ctx:
        tc = ctx.enter_context(tile.TileContext(nc))
        # outer pool: tensors whose lifetime spans projections AND attention
        sb = ctx.enter_context(tc.tile_pool(name="sb", bufs=1))

        kT_sb = [sb.tile([128, S], BF16, tag=f"kT{i}", name=f"kT{i}") for i in range(CCH)]
        # per-head q, zero-padded to full 128-row contraction: the PE cost of
        # a matmul is its moving-column count, so padding the contraction
        # with zero rows is cycle-free and exact (row-tiled K=64 pairs would
        # halve QK, but this walrus build cannot codegen tile_position).
        qTz_sb = [
            [sb.tile([128, SQ], BF16, tag=f"qz{i}_{p}", name=f"qz{i}_{p}")
             for p in range(2)]
            for i in range(CCH)
        ]
        # v in DoubleRow layout [p, head, kc-pair, VSTP]: tile jc2 stacks key
        # chunks 2*jc2 / 2*jc2+1 in the pair dim, fp8 -> PV contracts 256
        # keys per matmul.  VSTP pads the per-head stride so the pair-dim
        # stride (VSTP bytes) satisfies the dual-fp8 LDW %16 rule.
        VSTP = 80
        v_dr = [sb.tile([128, H, 2, VSTP], F8, tag=f"v{j}", name=f"v{j}")
                for j in range(NJC // 2)]
        attn_sb = [sb.tile([128, SQ], BF16, tag=f"attn{p}", name=f"attn{p}")
                   for p in range(NPAIR)]
        ones_sb = sb.tile([128, DH], BF16, tag="ones", name="ones")
        nc.vector.memset(ones_sb[:], 1.0)
        ones32_sb = sb.tile([DH + 1, DH], F32, tag="ones32", name="ones32")
        nc.vector.memset(ones32_sb[:], 1.0)
        wo_sb = [sb.tile([128, C], BF16, tag=f"wo{p}", name=f"wo{p}")
                 for p in range(NPAIR)]
        # output-projection accumulator: partial sums over head pairs land
        # here as each pair's normalization completes, so the epilogue only
        # waits on the LAST pair instead of running 50 matmuls cold.
        oacc_sb = [sb.tile([128, SQ], F32, tag=f"oacc{e}", name=f"oacc{e}")
                   for e in range(CCH)]

        # ---------------- load + first projections ----------------
        # Minimal critical path to the first exp: wk/wq column-slices for
        # feature chunk dc0 + hsT tokens 0:1024, then kproj(0,jt0)/qproj(0).
        load = ctx.enter_context(tc.tile_pool(name="load", bufs=1))
        # whole-tensor tiles [128, chunk(5), free]: feature chunk c of the
        # [C, *] DRAM tensor lands at [:, c, :] via one rearranged-AP DMA
        # (DMA dispatch costs ~0.6us per instruction on an engine queue, so
        # fewer+larger transfers win the startup race).  DoubleRow pair j
        # is the [:, 2j:2j+2, :] slice; the 5th chunk rides plain fp8.
        hs_sb = load.tile([128, CCH, S], F8, tag="hs", name="hs")
        nc.sync.dma_start(
            hs_sb[:, :, 0:SQ],
            hsT[:, 0:SQ].rearrange("(c p) t -> p c t", c=CCH),
        )

        def dma_w(name, src, eng):
            w = load.tile([128, CCH, C], F8, tag=name, name=name)
            eng.dma_start(w[:, :, :], src[:, :].rearrange("(c p) m -> p c m", c=CCH))
            return w

        wk_sb = dma_w("wk", wkT, nc.sync)
        wq_sb = dma_w("wq", wqT, nc.sync)
        wv_sb = dma_w("wv", wvT, nc.sync)

        def emit_hsT_tail():
            # deferred until after the first exp so ScalarE's conservative
            # vector-clock waits don't cover this 2MB of DMA
            nc.sync.dma_start(
                hs_sb[:, :, SQ:S],
                hsT[:, SQ:S].rearrange("(c p) t -> p c t", c=CCH),
            )

        def emit_wo_dma():
            for p in range(NPAIR):
                nc.gpsimd.dma_start(wo_sb[p][:], woT[ts(p, 128), :])

        def emit_kproj(dc, jt, pool):
            ps = pool.tile([128, 512], F32, tag="pp", name="pp", bufs=2)
            for j in range(2):
                nc.tensor.matmul(
                    ps[:],
                    wk_sb[:, 2 * j : 2 * j + 2, ts(dc, 128)],
                    hs_sb[:, 2 * j : 2 * j + 2, ts(jt, 512)],
                    start=(j == 0),
                    stop=False,
                    perf_mode=DR,
                )
            nc.tensor.matmul(
                ps[:],
                wk_sb[:, 4, ts(dc, 128)],
                hs_sb[:, 4, ts(jt, 512)],
                start=False,
                stop=True,
            )
            nc.vector.tensor_copy(kT_sb[dc][:, ts(jt, 512)], ps[:])

        def emit_qproj(dc, it, pool):
            ps = pool.tile([128, 512], F32, tag="pp", name="pp", bufs=2)
            for j in range(2):
                nc.tensor.matmul(
                    ps[:],
                    wq_sb[:, 2 * j : 2 * j + 2, ts(dc, 128)],
                    hs_sb[:, 2 * j : 2 * j + 2, ts(it, 512)],
                    start=(j == 0),
                    stop=False,
                    perf_mode=DR,
                )
            nc.tensor.matmul(
                ps[:],
                wq_sb[:, 4, ts(dc, 128)],
                hs_sb[:, 4, ts(it, 512)],
                start=False,
                stop=True,
            )
            nc.vector.tensor_copy(qTz_sb[dc][0][0:DH, ts(it, 512)], ps[0:DH, :])
            nc.vector.tensor_copy(qTz_sb[dc][1][DH:128, ts(it, 512)], ps[DH:128, :])

        def emit_vproj(jc, pool):
            v3 = v_dr[jc // 2][:, :, jc % 2, :]  # [128, H, VSTP]
            nc.vector.memset(v3[:, :, DH : DH + 1], 1.0)
            for d0, dn in ((0, 512), (512, 128)):
                ps = pool.tile([128, 512], F32, tag="pp", name="pp", bufs=2)
                for j in range(2):
                    nc.tensor.matmul(
                        ps[:, 0:dn],
                        hs_sb[:, 2 * j : 2 * j + 2, ts(jc, 128)],
                        wv_sb[:, 2 * j : 2 * j + 2, d0 : d0 + dn],
                        start=(j == 0),
                        stop=False,
                        perf_mode=DR,
                    )
                nc.tensor.matmul(
                    ps[:, 0:dn],
                    hs_sb[:, 4, ts(jc, 128)],
                    wv_sb[:, 4, d0 : d0 + dn],
                    start=False,
                    stop=True,
                )
                nc.vector.tensor_copy(
                    v3[:, d0 // DH : (d0 + dn) // DH, 0:DH],
                    ps[:, 0:dn].rearrange("p (h x) -> p h x", x=DH),
                )

        with tc.tile_pool(name="pp0", bufs=2, space="PSUM") as pp0:
            for dc in range(CCH):
                nc.vector.memset(qTz_sb[dc][0][DH:128, :], 0.0)
                nc.vector.memset(qTz_sb[dc][1][0:DH, :], 0.0)
            emit_kproj(0, 0, pp0)
            for it in range(NIT):
                emit_qproj(0, it, pp0)
            emit_kproj(0, 1, pp0)

        # ---------------- attention phase ----------------
        with tc.tile_pool(name="ap", bufs=1, space="PSUM") as ap, \
             tc.tile_pool(name="pt", bufs=4) as pt_pool, \
             tc.tile_pool(name="ob", bufs=3) as ob, \
             tc.tile_pool(name="scratch", bufs=1) as scratch:

            def norm_stage1(hp, pvA, pvB, it):
                # Fast, frees the pv banks: copy raw pv rows + denom row to
                # SBUF, then DMA-reshape both denom rows [1,512] into a
                # [128,8] tile so ONE cheap 8-element-per-lane reciprocal
                # serves both heads (a [1,512] reciprocal costs 3.3us of
                # head-of-line DVE queue time; [128,8] costs ~0.1us).
                raws, dens = [], []
                for half, pv in ((0, pvA), (1, pvB)):
                    raw = scratch.tile([DH, 512], BF16, tag=f"raw{half}",
                                       name=f"raw{half}")
                    nc.vector.tensor_copy(raw[:], pv[0:DH, :])
                    den = scratch.tile([DH + 1, 512], F32, tag=f"den{half}",
                                       name=f"den{half}")
                    nc.vector.tensor_copy(den[DH : DH + 1, :],
                                          pv[DH : DH + 1, :])
                    raws.append(raw)
                    dens.append(den)
                den2 = scratch.tile([128, 8], F32, tag="den2", name="den2")
                for half in range(2):
                    nc.sync.dma_start(
                        den2[:, half * 4 : half * 4 + 4],
                        dens[half][DH : DH + 1, :],
                    )
                return (hp, it, raws, den2)

            def norm_stage2(state):
                # One batched reciprocal, DMA the rows back (f32).
                hp, it, raws, den2 = state
                rcp2 = scratch.tile([128, 8], F32, tag="rcp2", name="rcp2")
                nc.vector.reciprocal(rcp2[:], den2[:])
                rcps = []
                for half in range(2):
                    rcp = scratch.tile([DH + 1, 512], F32, tag=f"rcp{half}",
                                       name=f"rcp{half}")
                    nc.sync.dma_start(
                        rcp[DH : DH + 1, :],
                        rcp2[:, half * 4 : half * 4 + 4],
                    )
                    rcps.append(rcp)
                return (hp, it, raws, rcps)

            def norm_stage3(state):
                # rank-1 PE broadcast of 1/den across 64 partitions (DVE
                # cannot cross partitions), then DVE mult into attn pair.
                hp, it, raws, rcps = state
                isl = ts(it, 512)
                for half in range(2):
                    bc = ap.tile([128, 512], F32, tag="pp", name="pp", bufs=2)
                    nc.tensor.matmul(
                        bc[0:DH, :],
                        ones32_sb[DH : DH + 1, :],
                        rcps[half][DH : DH + 1, :],
                        start=True,
                        stop=True,
                    )
                    nc.vector.tensor_mul(
                        attn_sb[hp][half * DH : (half + 1) * DH, isl],
                        raws[half][:], bc[0:DH, :],
                    )

            def emit_opart(hp, it, ec):
                # one head pair's contribution to output chunk (ec, it),
                # accumulated into SBUF right after the pair's norm lands
                isl = ts(it, 512)
                ps = ap.tile([128, 512], F32, tag="pp", name="pp", bufs=2)
                nc.tensor.matmul(
                    ps[:],
                    wo_sb[hp][:, ts(ec, 128)],
                    attn_sb[hp][:, isl],
                    start=True,
                    stop=True,
                )
                if hp == 0:
                    nc.vector.tensor_copy(oacc_sb[ec][:, isl], ps[:])
                else:
                    nc.vector.tensor_add(oacc_sb[ec][:, isl],
                                         oacc_sb[ec][:, isl], ps[:])

            def emit_ofinal(it, ec):
                isl = ts(it, 512)
                rt = ob.tile([128, 512], F32, tag="rt", name="rt", bufs=2)
                nc.sync.dma_start(rt[:], res[ts(ec, 128), isl])
                ot = ob.tile([128, 512], F32, tag="ot", name="ot", bufs=2)
                nc.vector.tensor_add(ot[:], oacc_sb[ec][:, isl], rt[:])
                nc.sync.dma_start(out[ts(ec, 128), isl], ot[:])

            # background work queue: one item is popped per (kc) step.
            bg = []
            bg.extend((lambda jt=jt: emit_kproj(0, jt, ap)) for jt in range(2, NJT))

            pend_norm = None
            for hp in range(NPAIR):
                if hp + 1 < NPAIR:
                    # next pair's K/Q projections ride this pair's window
                    bg.extend(
                        (lambda jt=jt, dc=hp + 1: emit_kproj(dc, jt, ap))
                        for jt in range(NJT)
                    )
                    bg.extend(
                        (lambda q_it=q_it, dc=hp + 1: emit_qproj(dc, q_it, ap))
                        for q_it in range(NIT)
                    )
                for it in range(NIT):
                    isl = ts(it, 512)
                    pvA = ap.tile([DH + 1, 512], F32, tag="pvA", bufs=1, name="pvA")
                    pvB = ap.tile([DH + 1, 512], F32, tag="pvB", bufs=1, name="pvB")
                    prev, pend_norm = pend_norm, None
                    nst = {}
                    for jc in range(NJC):
                        sc = ap.tile([128, 1024], F32, tag="sc", bufs=2, name="sc")
                        nc.tensor.matmul(
                            sc[:, 0:512],
                            kT_sb[hp][:, ts(jc, 128)],
                            qTz_sb[hp][0][:, isl],
                            start=True,
                            stop=True,
                        )
                        nc.tensor.matmul(
                            sc[:, 512:1024],
                            kT_sb[hp][:, ts(jc, 128)],
                            qTz_sb[hp][1][:, isl],
                            start=True,
                            stop=True,
                        )
                        if jc % 2 == 0:
                            # [p, head(2), kc-pair(2), q]: fp8 probs laid out
                            # so PV can contract 256 keys via DoubleRow
                            pt = pt_pool.tile([128, 2, 2, 512], F8, tag="pt",
                                              name="pt")
                        if not (hp == 0 and it == 0) and jc % 4 == 1:
                            nc.vector.tensor_scalar(
                                pt[:, :, jc % 2, :].bitcast(mybir.dt.int8),
                                sc[:].rearrange("p (h q) -> p h q", q=512),
                                EXP_A8,
                                EXP_B8,
                                mybir.AluOpType.mult,
                                mybir.AluOpType.add,
                            )
                        else:
                            nc.scalar.activation(
                                pt[:, :, jc % 2, :],
                                sc[:].rearrange("p (h q) -> p h q", q=512),
                                mybir.ActivationFunctionType.Exp,
                                bias=0.0, scale=SCALE,
                            )
                        # staged normalization + output-projection of the
                        # PREVIOUS (pair, it): stages are spaced so DMA
                        # round-trips complete before their consumer enters
                        # the strict-FIFO DVE queue (no head-of-line
                        # blocking of the projection PSUM-evacuation
                        # copies), and the opart/ofinal work dribbles out
                        # one chunk per step.
                        if prev is not None:
                            phr, _, _, pit = prev
                            if jc == 0:
                                nst[1] = norm_stage1(*prev)
                            elif jc == 6:
                                nst[2] = norm_stage2(nst[1])
                            elif jc == 12:
                                norm_stage3(nst[2])
                            elif 13 <= jc < 13 + CCH:
                                emit_opart(phr, pit, jc - 13)
                            elif (phr == NPAIR - 1
                                  and 18 <= jc < 18 + CCH):
                                emit_ofinal(pit, jc - 18)
                        if hp == 0 and it == 0:
                            if jc == 0:
                                emit_hsT_tail()
                            if jc == 1:
                                emit_wo_dma()
                            # V projection rides inside the first pair's
                            # window, each chunk just ahead of its PV
                            emit_vproj(jc, ap)
                            if jc % 3 == 2 and bg:
                                bg.pop(0)()
                        elif hp == 0 and it == 1:
                            # drain remaining kproj/qproj for the next pair
                            if bg:
                                bg.pop(0)()
                            if jc % 2 == 0 and bg:
                                bg.pop(0)()
                        elif bg:
                            bg.pop(0)()
                        if jc % 2 == 1:
                            jc2 = jc // 2
                            for half, pv in ((0, pvA), (1, pvB)):
                                h = 2 * hp + half
                                nc.tensor.matmul(
                                    pv[:],
                                    v_dr[jc2][:, h, :, 0:VST],
                                    pt[:, half, :, :],
                                    start=(jc2 == 0),
                                    stop=(jc2 == NJC // 2 - 1),
                                    perf_mode=DR,
                                )
                    pend_norm = (hp, pvA, pvB, it)
                # barrier: next pair's kT/qT must be fully emitted before
                # its first QK reads them
                while bg:
                    bg.pop(0)()
            s1 = norm_stage1(*pend_norm)
            s2 = norm_stage2(s1)
            norm_stage3(s2)
            for ec in range(CCH):
                emit_opart(NPAIR - 1, 1, ec)
            for ec in range(CCH):
                emit_ofinal(1, ec)

    _spill_matmul_waits(nc)
    return nc


# walrus embedded-sync-wait capacity per BIR opcode.  Matmult holds a
# single wait; excess waits hoist onto the paired Ldweights (in-order
# issue on PE makes that equivalent).  Other compute ops spill onto
# EventSemaphore carrier instructions inserted just before them on the
# same engine.  DMACopy / Drain / EventSemaphore handle many waits
# natively (bacc emits such itself) and are left alone.
_WAIT_CAPS = {
    "InstMatmult": 1,
    "InstLdweights": 1,
    "InstActivation": 1,
    "InstReciprocal": 1,
    "InstTensorTensor": 1,
    "InstTensorCopy": 1,
    "InstTensorScalarPtr": 1,
    "InstTensorReduce": 1,
    "InstMemset": 1,
    "InstDMACopy": 1,
    "InstDrain": 1,
    "InstCustomDveAnt": 1,
}
_ES_CAP = 2  # waits per EventSemaphore carrier (walrus: <=2 waits, <=1 update)


def _spill_matmul_waits(nc: bass.Bass) -> None:
    spill_id = [0]

    def carriers(excess, engine):
        out = []
        for i in range(0, len(excess), _ES_CAP):
            es = mybir.InstEventSemaphore(
                name=f"wait-spill-{spill_id[0]}", ins=[], outs=[]
            )
            spill_id[0] += 1
            es.engine = engine
            es.sync_info = mybir.SyncInfo(
                on_wait=excess[i : i + _ES_CAP], on_update=[]
            )
            out.append(es)
        return out

    for f in nc.m.functions:
        for blk in f.blocks:
            insts = blk.instructions
            i = 0
            while i < len(insts):
                inst = insts[i]
                tn = type(inst).__name__
                cap = _WAIT_CAPS.get(tn)
                si = inst.sync_info
                if cap is None or si is None or len(si.on_wait) <= cap:
                    i += 1
                    continue
                w = list(si.on_wait)
                if tn == "InstMatmult" and cap == 1:
                    # Keep the latest-satisfied dependency (the ACT-produced
                    # operand, e.g. probs from exp) embedded on the matmul and
                    # hoist early ones onto the Ldweights: a wait on the LDW
                    # blocks its background prefetch and serializes ~50ns of
                    # weight-load into every PV matmul.
                    acts = [x for x in w if "Activation" in (x.ant_name or "")]
                    if acts:
                        keep = [acts[-1]]
                        excess = [x for x in w if x is not acts[-1]]
                    else:
                        keep, excess = w[-cap:], w[:-cap]
                else:
                    keep, excess = w[-cap:], w[:-cap]
                prev = insts[i - 1] if i > 0 else None
                if (
                    tn == "InstMatmult"
                    and prev is not None
                    and type(prev).__name__ == "InstLdweights"
                    and len(((prev.sync_info and prev.sync_info.on_wait) or []))
                    + len(excess) <= 1
                ):
                    psi = prev.sync_info
                    pw = list(psi.on_wait) if psi is not None else []
                    pu = list(psi.on_update) if psi is not None else []
                    prev.sync_info = mybir.SyncInfo(on_wait=pw + excess, on_update=pu)
                else:
                    new = carriers(excess, inst.engine)
                    insts[i:i] = new
                    i += len(new)
                inst.sync_info = mybir.SyncInfo(
                    on_wait=keep, on_update=list(si.on_update)
                )
                i += 1


_CACHED_NC = None


def get_nc() -> bass.Bass:
    global _CACHED_NC
    if _CACHED_NC is None:
        _CACHED_NC = build_nc()
    return _CACHED_NC


def make_in_maps(hidden_states, Wq, Wk, Wv, Wo, b_out):
    hs = np.asarray(hidden_states, dtype=np.float32)
    bf = ml_dtypes.bfloat16
    f8 = ml_dtypes.float8_e4m3

    def to_f8(x):
        return np.clip(x, -224.0, 224.0).astype(f8)

    # hs scaled by 1/8 and W by 8 (exact products) so both sit in fp8e4's
    # normal range (w_std=0.02 would otherwise be mostly denormal).
    wqT = to_f8(np.ascontiguousarray(np.asarray(Wq, np.float32).T) * 8.0)
    wkT = to_f8(np.ascontiguousarray(np.asarray(Wk, np.float32).T) * 8.0)
    wvT = to_f8(np.ascontiguousarray(np.asarray(Wv, np.float32).T) * 8.0)
    woT = np.ascontiguousarray(np.asarray(Wo, np.float32).T).astype(bf)
    bias = np.asarray(b_out, np.float32).reshape(C, 1)
    in_maps = []
    for c in range(NCORES):
        b, g = divmod(c, GROUP)
        i0 = g * SQ
        hsTb = hs[b].T  # [C, S]
        in_maps.append(
            {
                "hsT": to_f8(np.ascontiguousarray(np.roll(hsTb, -i0, axis=1)) * 0.125),
                "res": np.ascontiguousarray(hsTb[:, i0 : i0 + SQ]) + bias,
                "wqT": wqT,
                "wkT": wkT,
                "wvT": wvT,
                "woT": woT,
            }
        )
    return in_maps


def assemble(results) -> np.ndarray:
    y = np.empty((B, S, C), np.float32)
    for c in range(NCORES):
        b, g = divmod(c, GROUP)
        i0 = g * SQ
        y[b, i0 : i0 + SQ, :] = np.asarray(results[c]["out"], np.float32).T
    return y


def kernel(**inputs) -> np.ndarray:
    from concourse.bass_utils import run_bass_kernel_spmd

    nc = get_nc()
    in_maps = make_in_maps(**inputs)
    res = run_bass_kernel_spmd(nc, in_maps, list(range(NCORES)))
    return assemble(res.results)


if __name__ == "__main__":
    import reference

    inputs = {k: np.asarray(v) for k, v in reference.setup_inputs().items()}
    got = kernel(**inputs)
    want = np.asarray(reference.reference(**inputs))
    err = np.linalg.norm(got - want) / np.linalg.norm(want)
    print("Relative error:", err)


# revision 47
# speedup vs baseline: 1.0127x; 1.0127x over previous
"""Multi-head attention (AttnProcessor2_0) on 8 TRN2 NeuronCores.

Problem: B=2, S=4096, C=640, H=10, Dh=64.
  q/k/v = hs @ W{q,k,v}.T ; per-head scores = q k^T / 8 ; softmax ;
  out = probs v ; y = out @ Wo.T + b_out + hs

Sharding (no collectives): core c -> batch b=c//4, query block g=c%4
(1024 queries).  Each core recomputes full K/V for its batch (head-dim
on partitions), computes its own S/4 x S attention block, output
projection, bias+residual.  Host passes hidden states TRANSPOSED and
ROLLED by the query offset so the same SPMD program works on every
core (softmax+PV are permutation-invariant along the key axis).

Device layout (feature-on-partition, token-on-free):
  kT [640, 4096] (5 chunks of 128 = 2 heads each)
  qT [5][128, 1024] pair layout: head 2i on partitions 0:64, head
     2i+1 on 64:128 (natural projection output, no zero padding)
  QK row-tiled pairs: two K=64 matmuls (row groups 0-1 / 2-3 via
     base-partition slicing) run CONCURRENTLY on the PE -> both heads'
     [128 keys x 512 q] score blocks in ~512 cycles instead of 1024.
  v  [4096, 650] (65-stride per head: 64 cols + ones col -> softmax
     denominators fall out of the PV matmul as PSUM row 64)
  probs: scoresT in PSUM -> ScalarE exp -> bf16 SBUF
  normalization: reciprocal_approx_fast of denom row, rank-1 PE outer
     product to broadcast across partitions, DVE mult into pair-packed
     attn tiles [128, 1024] (head 2i rows 0:64, head 2i+1 rows 64:128)
  oproj: pair-packed -> 5 matmuls of full K=128 contraction per
     (128-out-chunk, 512q), Wo tiles loaded as direct [128,128] DMAs.
Loop order: head-pair OUTER, query-tile inner -> background projection
windows are 2x wider; K/Q/V projections and oproj(it0) ride the PE
slack inside the attention loop (2 spare PSUM banks, tag "pp").
All matmuls bf16 (f32 PSUM accumulation).
"""

import sys

if "/opt/trn_rl_repo" not in sys.path:
    sys.path.insert(0, "/opt/trn_rl_repo")

from contextlib import ExitStack

import ml_dtypes
import numpy as np

import concourse.bass as bass
import concourse.tile as tile
from concourse import mybir
from concourse.bass import ts

BF16 = mybir.dt.bfloat16
F32 = mybir.dt.float32
F8 = mybir.dt.float8e4
DR = mybir.MatmulPerfMode.DoubleRow

B, S, C = 2, 4096, 640
H, DH = 10, 64
NCORES = 8
GROUP = 4  # cores per batch element
SQ = S // GROUP  # 1024 queries per core
SCALE = 0.125  # 1/sqrt(64)
CCH = C // 128  # 5 feature chunks (2 heads each)
NJT = S // 512  # 8 key tiles for K proj
NJC = S // 128  # 32 key chunks for attention
NIT = SQ // 512  # 2 query tiles
VST = DH + 1  # 65: per-head stride in v tiles (ones col appended)
NPAIR = H // 2  # 5 head pairs

# DVE-offloaded exp: fp8e4m3 bits of exp(SCALE*x) are approximately the
# affine x*A8+B8 (Schraudolph bit trick), so a single DVE tensor_scalar
# with int8 output replaces a ScalarE ACTIVATE for ~25% of the softmax --
# the two engines split the 41.9M-element exp stream.  The +-3% sawtooth
# is random across keys and self-normalizes through the denominators.
EXP_A8 = SCALE * 8.0 / np.log(2.0)
EXP_B8 = 55.54


def build_nc() -> bass.Bass:
    nc = bass.Bass()
    # hsT/W{q,k,v} travel as fp8e4 with hs pre-scaled by 1/8 and W by 8 on
    # the host (products exact): both land in fp8's normal range, and the
    # K/Q/V projections run DoubleRow (256-row virtual contraction, ~1.77x).
    hsT = nc.declare_dram_parameter("hsT", [C, S], F8, isOutput=False)
    res = nc.declare_dram_parameter("res", [C, SQ], F32, isOutput=False)
    wqT = nc.declare_dram_parameter("wqT", [C, C], F8, isOutput=False)
    wkT = nc.declare_dram_parameter("wkT", [C, C], F8, isOutput=False)
    wvT = nc.declare_dram_parameter("wvT", [C, C], F8, isOutput=False)
    woT = nc.declare_dram_parameter("woT", [C, C], BF16, isOutput=False)
    out = nc.declare_dram_parameter("out", [C, SQ], F32, isOutput=True)

    with ExitStack() as ctx:
        tc = ctx.enter_context(tile.TileContext(nc))
        # outer pool: tensors whose lifetime spans projections AND attention
        sb = ctx.enter_context(tc.tile_pool(name="sb", bufs=1))

        kT_sb = [sb.tile([128, S], BF16, tag=f"kT{i}", name=f"kT{i}") for i in range(CCH)]
        # per-head q, zero-padded to full 128-row contraction: the PE cost of
        # a matmul is its moving-column count, so padding the contraction
        # with zero rows is cycle-free and exact (row-tiled K=64 pairs would
        # halve QK, but this walrus build cannot codegen tile_position).
        qTz_sb = [
            [sb.tile([128, SQ], BF16, tag=f"qz{i}_{p}", name=f"qz{i}_{p}")
             for p in range(2)]
            for i in range(CCH)
        ]
        # v in DoubleRow layout [p, head, kc-pair, VSTP]: tile jc2 stacks key
        # chunks 2*jc2 / 2*jc2+1 in the pair dim, fp8 -> PV contracts 256
        # keys per matmul.  VSTP pads the per-head stride so the pair-dim
        # stride (VSTP bytes) satisfies the dual-fp8 LDW %16 rule.
        VSTP = 80
        v_dr = [sb.tile([128, H, 2, VSTP], F8, tag=f"v{j}", name=f"v{j}")
                for j in range(NJC // 2)]
        attn_sb = [sb.tile([128, SQ], BF16, tag=f"attn{p}", name=f"attn{p}")
                   for p in range(NPAIR)]
        ones_sb = sb.tile([128, DH], BF16, tag="ones", name="ones")
        nc.vector.memset(ones_sb[:], 1.0)
        ones32_sb = sb.tile([DH + 1, DH], F32, tag="ones32", name="ones32")
        nc.vector.memset(ones32_sb[:], 1.0)
        wo_sb = [sb.tile([128, C], BF16, tag=f"wo{p}", name=f"wo{p}")
                 for p in range(NPAIR)]
        # output-projection accumulator: partial sums over head pairs land
        # here as each pair's normalization completes, so the epilogue only
        # waits on the LAST pair instead of running 50 matmuls cold.
        oacc_sb = [sb.tile([128, SQ], F32, tag=f"oacc{e}", name=f"oacc{e}")
                   for e in range(CCH)]

        # ---------------- load + first projections ----------------
        # Minimal critical path to the first exp: wk/wq column-slices for
        # feature chunk dc0 + hsT tokens 0:1024, then kproj(0,jt0)/qproj(0).
        load = ctx.enter_context(tc.tile_pool(name="load", bufs=1))
        # whole-tensor tiles [128, chunk(5), free]: feature chunk c of the
        # [C, *] DRAM tensor lands at [:, c, :] via one rearranged-AP DMA
        # (DMA dispatch costs ~0.6us per instruction on an engine queue, so
        # fewer+larger transfers win the startup race).  DoubleRow pair j
        # is the [:, 2j:2j+2, :] slice; the 5th chunk rides plain fp8.
        hs_sb = load.tile([128, CCH, S], F8, tag="hs", name="hs")
        nc.sync.dma_start(
            hs_sb[:, :, 0:SQ],
            hsT[:, 0:SQ].rearrange("(c p) t -> p c t", c=CCH),
        )

        def dma_w(name, src, eng):
            w = load.tile([128, CCH, C], F8, tag=name, name=name)
            eng.dma_start(w[:, :, :], src[:, :].rearrange("(c p) m -> p c m", c=CCH))
            return w

        wk_sb = dma_w("wk", wkT, nc.sync)
        wq_sb = dma_w("wq", wqT, nc.sync)
        wv_sb = dma_w("wv", wvT, nc.sync)

        def emit_hsT_tail():
            # deferred until after the first exp so ScalarE's conservative
            # vector-clock waits don't cover this 2MB of DMA
            nc.sync.dma_start(
                hs_sb[:, :, SQ:S],
                hsT[:, SQ:S].rearrange("(c p) t -> p c t", c=CCH),
            )

        def emit_wo_dma():
            for p in range(NPAIR):
                nc.gpsimd.dma_start(wo_sb[p][:], woT[ts(p, 128), :])

        def emit_kproj(dc, jt, pool):
            ps = pool.tile([128, 512], F32, tag="pp", name="pp", bufs=2)
            for j in range(2):
                nc.tensor.matmul(
                    ps[:],
                    wk_sb[:, 2 * j : 2 * j + 2, ts(dc, 128)],
                    hs_sb[:, 2 * j : 2 * j + 2, ts(jt, 512)],
                    start=(j == 0),
                    stop=False,
                    perf_mode=DR,
                )
            nc.tensor.matmul(
                ps[:],
                wk_sb[:, 4, ts(dc, 128)],
                hs_sb[:, 4, ts(jt, 512)],
                start=False,
                stop=True,
            )
            nc.vector.tensor_copy(kT_sb[dc][:, ts(jt, 512)], ps[:])

        def emit_qproj(dc, it, pool):
            ps = pool.tile([128, 512], F32, tag="pp", name="pp", bufs=2)
            for j in range(2):
                nc.tensor.matmul(
                    ps[:],
                    wq_sb[:, 2 * j : 2 * j + 2, ts(dc, 128)],
                    hs_sb[:, 2 * j : 2 * j + 2, ts(it, 512)],
                    start=(j == 0),
                    stop=False,
                    perf_mode=DR,
                )
            nc.tensor.matmul(
                ps[:],
                wq_sb[:, 4, ts(dc, 128)],
                hs_sb[:, 4, ts(it, 512)],
                start=False,
                stop=True,
            )
            nc.vector.tensor_copy(qTz_sb[dc][0][0:DH, ts(it, 512)], ps[0:DH, :])
            nc.vector.tensor_copy(qTz_sb[dc][1][DH:128, ts(it, 512)], ps[DH:128, :])

        def emit_vproj(jc, pool):
            v3 = v_dr[jc // 2][:, :, jc % 2, :]  # [128, H, VSTP]
            nc.vector.memset(v3[:, :, DH : DH + 1], 1.0)
            for d0, dn in ((0, 512), (512, 128)):
                ps = pool.tile([128, 512], F32, tag="pp", name="pp", bufs=2)
                for j in range(2):
                    nc.tensor.matmul(
                        ps[:, 0:dn],
                        hs_sb[:, 2 * j : 2 * j + 2, ts(jc, 128)],
                        wv_sb[:, 2 * j : 2 * j + 2, d0 : d0 + dn],
                        start=(j == 0),
                        stop=False,
                        perf_mode=DR,
                    )
                nc.tensor.matmul(
                    ps[:, 0:dn],
                    hs_sb[:, 4, ts(jc, 128)],
                    wv_sb[:, 4, d0 : d0 + dn],
                    start=False,
                    stop=True,
                )
                nc.vector.tensor_copy(
                    v3[:, d0 // DH : (d0 + dn) // DH, 0:DH],
                    ps[:, 0:dn].rearrange("p (h x) -> p h x", x=DH),
                )

        with tc.tile_pool(name="pp0", bufs=2, space="PSUM") as pp0:
            for dc in range(CCH):
                nc.vector.memset(qTz_sb[dc][0][DH:128, :], 0.0)
                nc.vector.memset(qTz_sb[dc][1][0:DH, :], 0.0)
            emit_kproj(0, 0, pp0)
            for it in range(NIT):
                emit_qproj(0, it, pp0)
            emit_kproj(0, 1, pp0)

        # ---------------- attention phase ----------------
        with tc.tile_pool(name="ap", bufs=1, space="PSUM") as ap, \
             tc.tile_pool(name="pt", bufs=4) as pt_pool, \
             tc.tile_pool(name="ob", bufs=3) as ob, \
             tc.tile_pool(name="scratch", bufs=1) as scratch:

            def norm_stage1(hp, pvA, pvB, it):
                # Fast, frees the pv banks: copy raw pv rows + denom row to
                # SBUF, then DMA-reshape both denom rows [1,512] into a
                # [128,8] tile so ONE cheap 8-element-per-lane reciprocal
                # serves both heads (a [1,512] reciprocal costs 3.3us of
                # head-of-line DVE queue time; [128,8] costs ~0.1us).
                raws, dens = [], []
                for half, pv in ((0, pvA), (1, pvB)):
                    raw = scratch.tile([DH, 512], BF16, tag=f"raw{half}",
                                       name=f"raw{half}")
                    nc.vector.tensor_copy(raw[:], pv[0:DH, :])
                    den = scratch.tile([DH + 1, 512], F32, tag=f"den{half}",
                                       name=f"den{half}")
                    nc.vector.tensor_copy(den[DH : DH + 1, :],
                                          pv[DH : DH + 1, :])
                    raws.append(raw)
                    dens.append(den)
                den2 = scratch.tile([128, 8], F32, tag="den2", name="den2")
                for half in range(2):
                    nc.sync.dma_start(
                        den2[:, half * 4 : half * 4 + 4],
                        dens[half][DH : DH + 1, :],
                    )
                return (hp, it, raws, den2)

            def norm_stage2(state):
                # One batched reciprocal, cast to bf16 (tiny), DMA rows back.
                hp, it, raws, den2 = state
                rcp2 = scratch.tile([128, 8], F32, tag="rcp2", name="rcp2")
                nc.vector.reciprocal(rcp2[:], den2[:])
                rcp2b = scratch.tile([128, 8], BF16, tag="rcp2b", name="rcp2b")
                with nc.allow_low_precision(reason="softmax recip bf16"):
                    nc.vector.tensor_copy(rcp2b[:], rcp2[:])
                rcps = []
                for half in range(2):
                    rcp = scratch.tile([DH + 1, 512], BF16, tag=f"rcp{half}",
                                       name=f"rcp{half}")
                    nc.sync.dma_start(
                        rcp[DH : DH + 1, :],
                        rcp2b[:, half * 4 : half * 4 + 4],
                    )
                    rcps.append(rcp)
                return (hp, it, raws, rcps)

            def norm_stage3(state):
                # rank-1 PE broadcast of 1/den across 64 partitions (DVE
                # cannot cross partitions), then DVE mult into attn pair.
                hp, it, raws, rcps = state
                isl = ts(it, 512)
                for half in range(2):
                    bc = ap.tile([128, 512], F32, tag="pp", name="pp", bufs=2)
                    nc.tensor.matmul(
                        bc[0:DH, :],
                        ones_sb[DH : DH + 1, :],
                        rcps[half][DH : DH + 1, :],
                        start=True,
                        stop=True,
                    )
                    nc.vector.tensor_mul(
                        attn_sb[hp][half * DH : (half + 1) * DH, isl],
                        raws[half][:], bc[0:DH, :],
                    )

            def emit_opart(hp, it, ec):
                # one head pair's contribution to output chunk (ec, it),
                # accumulated into SBUF right after the pair's norm lands
                isl = ts(it, 512)
                ps = ap.tile([128, 512], F32, tag="pp", name="pp", bufs=2)
                nc.tensor.matmul(
                    ps[:],
                    wo_sb[hp][:, ts(ec, 128)],
                    attn_sb[hp][:, isl],
                    start=True,
                    stop=True,
                )
                if hp == 0:
                    nc.vector.tensor_copy(oacc_sb[ec][:, isl], ps[:])
                else:
                    nc.vector.tensor_add(oacc_sb[ec][:, isl],
                                         oacc_sb[ec][:, isl], ps[:])

            def emit_ofinal(it, ec):
                isl = ts(it, 512)
                rt = ob.tile([128, 512], F32, tag="rt", name="rt", bufs=2)
                nc.sync.dma_start(rt[:], res[ts(ec, 128), isl])
                ot = ob.tile([128, 512], F32, tag="ot", name="ot", bufs=2)
                nc.vector.tensor_add(ot[:], oacc_sb[ec][:, isl], rt[:])
                nc.sync.dma_start(out[ts(ec, 128), isl], ot[:])

            # background work queue: one item is popped per (kc) step.
            bg = []
            bg.extend((lambda jt=jt: emit_kproj(0, jt, ap)) for jt in range(2, NJT))

            pend_norm = None
            for hp in range(NPAIR):
                if hp + 1 < NPAIR:
                    # next pair's K/Q projections ride this pair's window
                    bg.extend(
                        (lambda jt=jt, dc=hp + 1: emit_kproj(dc, jt, ap))
                        for jt in range(NJT)
                    )
                    bg.extend(
                        (lambda q_it=q_it, dc=hp + 1: emit_qproj(dc, q_it, ap))
                        for q_it in range(NIT)
                    )
                for it in range(NIT):
                    isl = ts(it, 512)
                    pvA = ap.tile([DH + 1, 512], F32, tag="pvA", bufs=1, name="pvA")
                    pvB = ap.tile([DH + 1, 512], F32, tag="pvB", bufs=1, name="pvB")
                    prev, pend_norm = pend_norm, None
                    nst = {}
                    for jc in range(NJC):
                        sc = ap.tile([128, 1024], F32, tag="sc", bufs=2, name="sc")
                        nc.tensor.matmul(
                            sc[:, 0:512],
                            kT_sb[hp][:, ts(jc, 128)],
                            qTz_sb[hp][0][:, isl],
                            start=True,
                            stop=True,
                        )
                        nc.tensor.matmul(
                            sc[:, 512:1024],
                            kT_sb[hp][:, ts(jc, 128)],
                            qTz_sb[hp][1][:, isl],
                            start=True,
                            stop=True,
                        )
                        if jc % 2 == 0:
                            # [p, head(2), kc-pair(2), q]: fp8 probs laid out
                            # so PV can contract 256 keys via DoubleRow
                            pt = pt_pool.tile([128, 2, 2, 512], F8, tag="pt",
                                              name="pt")
                        if not (hp == 0 and it == 0) and jc % 4 == 1:
                            nc.vector.tensor_scalar(
                                pt[:, :, jc % 2, :].bitcast(mybir.dt.int8),
                                sc[:].rearrange("p (h q) -> p h q", q=512),
                                EXP_A8,
                                EXP_B8,
                                mybir.AluOpType.mult,
                                mybir.AluOpType.add,
                            )
                        else:
                            nc.scalar.activation(
                                pt[:, :, jc % 2, :],
                                sc[:].rearrange("p (h q) -> p h q", q=512),
                                mybir.ActivationFunctionType.Exp,
                                bias=0.0, scale=SCALE,
                            )
                        # staged normalization + output-projection of the
                        # PREVIOUS (pair, it): stages are spaced so DMA
                        # round-trips complete before their consumer enters
                        # the strict-FIFO DVE queue (no head-of-line
                        # blocking of the projection PSUM-evacuation
                        # copies), and the opart/ofinal work dribbles out
                        # one chunk per step.
                        if prev is not None:
                            phr, _, _, pit = prev
                            if jc == 0:
                                nst[1] = norm_stage1(*prev)
                            elif jc == 6:
                                nst[2] = norm_stage2(nst[1])
                            elif jc == 12:
                                norm_stage3(nst[2])
                            elif 13 <= jc < 13 + CCH:
                                emit_opart(phr, pit, jc - 13)
                            elif (phr == NPAIR - 1
                                  and 18 <= jc < 18 + CCH):
                                emit_ofinal(pit, jc - 18)
                        if hp == 0 and it == 0:
                            if jc == 0:
                                emit_hsT_tail()
                            if jc == 1:
                                emit_wo_dma()
                            # V projection rides inside the first pair's
                            # window, each chunk just ahead of its PV
                            emit_vproj(jc, ap)
                            if jc % 3 == 2 and bg:
                                bg.pop(0)()
                        elif hp == 0 and it == 1:
                            # drain remaining kproj/qproj for the next pair
                            if bg:
                                bg.pop(0)()
                            if jc % 2 == 0 and bg:
                                bg.pop(0)()
                        elif bg:
                            bg.pop(0)()
                        if jc % 2 == 1:
                            jc2 = jc // 2
                            for half, pv in ((0, pvA), (1, pvB)):
                                h = 2 * hp + half
                                nc.tensor.matmul(
                                    pv[:],
                                    v_dr[jc2][:, h, :, 0:VST],
                                    pt[:, half, :, :],
                                    start=(jc2 == 0),
                                    stop=(jc2 == NJC // 2 - 1),
                                    perf_mode=DR,
                                )
                    pend_norm = (hp, pvA, pvB, it)
                # barrier: next pair's kT/qT must be fully emitted before
                # its first QK reads them
                while bg:
                    bg.pop(0)()
            s1 = norm_stage1(*pend_norm)
            s2 = norm_stage2(s1)
            norm_stage3(s2)
            for ec in range(CCH):
                emit_opart(NPAIR - 1, 1, ec)
            for ec in range(CCH):
                emit_ofinal(1, ec)

    _dedupe_ldweights(nc)
    _spill_matmul_waits(nc)
    return nc


def _dedupe_ldweights(nc: bass.Bass) -> None:
    """Drop a LDWEIGHTS that reloads the stationary operand the PE already
    holds (e.g. the two QK matmuls of a head pair share one kT chunk).
    Matmuls do not invalidate loaded weights; any other PE-engine
    instruction conservatively resets the tracked state."""
    for f in nc.m.functions:
        for blk in f.blocks:
            keep = []
            prev_sig = None
            mm_engine = None
            for inst in blk.instructions:
                tn = type(inst).__name__
                if tn == "InstMatmult":
                    mm_engine = inst.engine
                    break
            for inst in blk.instructions:
                tn = type(inst).__name__
                if tn == "InstLdweights":
                    si = inst.sync_info
                    clean = si is None or (not si.on_wait and not si.on_update)
                    sig = repr(inst.ins[0]) + repr(getattr(inst, "perf_mode", None))
                    if clean and sig == prev_sig:
                        continue
                    prev_sig = sig
                elif tn in ("InstMatmult", "InstEventSemaphore"):
                    pass
                elif getattr(inst, "engine", None) == mm_engine:
                    prev_sig = None
                keep.append(inst)
            blk.instructions[:] = keep


# walrus embedded-sync-wait capacity per BIR opcode.  Matmult holds a
# single wait; excess waits hoist onto the paired Ldweights (in-order
# issue on PE makes that equivalent).  Other compute ops spill onto
# EventSemaphore carrier instructions inserted just before them on the
# same engine.  DMACopy / Drain / EventSemaphore handle many waits
# natively (bacc emits such itself) and are left alone.
_WAIT_CAPS = {
    "InstMatmult": 1,
    "InstLdweights": 1,
    "InstActivation": 1,
    "InstReciprocal": 1,
    "InstTensorTensor": 1,
    "InstTensorCopy": 1,
    "InstTensorScalarPtr": 1,
    "InstTensorReduce": 1,
    "InstMemset": 1,
    "InstDMACopy": 1,
    "InstDrain": 1,
    "InstCustomDveAnt": 1,
}
_ES_CAP = 2  # waits per EventSemaphore carrier (walrus: <=2 waits, <=1 update)


def _spill_matmul_waits(nc: bass.Bass) -> None:
    spill_id = [0]

    def carriers(excess, engine):
        out = []
        for i in range(0, len(excess), _ES_CAP):
            es = mybir.InstEventSemaphore(
                name=f"wait-spill-{spill_id[0]}", ins=[], outs=[]
            )
            spill_id[0] += 1
            es.engine = engine
            es.sync_info = mybir.SyncInfo(
                on_wait=excess[i : i + _ES_CAP], on_update=[]
            )
            out.append(es)
        return out

    for f in nc.m.functions:
        for blk in f.blocks:
            insts = blk.instructions
            i = 0
            while i < len(insts):
                inst = insts[i]
                tn = type(inst).__name__
                cap = _WAIT_CAPS.get(tn)
                si = inst.sync_info
                if cap is None or si is None or len(si.on_wait) <= cap:
                    i += 1
                    continue
                w = list(si.on_wait)
                if tn == "InstMatmult" and cap == 1:
                    # Keep the latest-satisfied dependency (the ACT-produced
                    # operand, e.g. probs from exp) embedded on the matmul and
                    # hoist early ones onto the Ldweights: a wait on the LDW
                    # blocks its background prefetch and serializes ~50ns of
                    # weight-load into every PV matmul.
                    acts = [x for x in w if "Activation" in (x.ant_name or "")]
                    if acts:
                        keep = [acts[-1]]
                        excess = [x for x in w if x is not acts[-1]]
                    else:
                        keep, excess = w[-cap:], w[:-cap]
                else:
                    keep, excess = w[-cap:], w[:-cap]
                prev = insts[i - 1] if i > 0 else None
                if (
                    tn == "InstMatmult"
                    and prev is not None
                    and type(prev).__name__ == "InstLdweights"
                    and len(((prev.sync_info and prev.sync_info.on_wait) or []))
                    + len(excess) <= 1
                ):
                    psi = prev.sync_info
                    pw = list(psi.on_wait) if psi is not None else []
                    pu = list(psi.on_update) if psi is not None else []
                    prev.sync_info = mybir.SyncInfo(on_wait=pw + excess, on_update=pu)
                else:
                    new = carriers(excess, inst.engine)
                    insts[i:i] = new
                    i += len(new)
                inst.sync_info = mybir.SyncInfo(
                    on_wait=keep, on_update=list(si.on_update)
                )
                i += 1


_CACHED_NC = None


def get_nc() -> bass.Bass:
    global _CACHED_NC
    if _CACHED_NC is None:
        _CACHED_NC = build_nc()
    return _CACHED_NC


def make_in_maps(hidden_states, Wq, Wk, Wv, Wo, b_out):
    hs = np.asarray(hidden_states, dtype=np.float32)
    bf = ml_dtypes.bfloat16
    f8 = ml_dtypes.float8_e4m3

    def to_f8(x):
        return np.clip(x, -224.0, 224.0).astype(f8)

    # hs scaled by 1/8 and W by 8 (exact products) so both sit in fp8e4's
    # normal range (w_std=0.02 would otherwise be mostly denormal).
    wqT = to_f8(np.ascontiguousarray(np.asarray(Wq, np.float32).T) * 8.0)
    wkT = to_f8(np.ascontiguousarray(np.asarray(Wk, np.float32).T) * 8.0)
    wvT = to_f8(np.ascontiguousarray(np.asarray(Wv, np.float32).T) * 8.0)
    woT = np.ascontiguousarray(np.asarray(Wo, np.float32).T).astype(bf)
    bias = np.asarray(b_out, np.float32).reshape(C, 1)
    in_maps = []
    for c in range(NCORES):
        b, g = divmod(c, GROUP)
        i0 = g * SQ
        hsTb = hs[b].T  # [C, S]
        in_maps.append(
            {
                "hsT": to_f8(np.ascontiguousarray(np.roll(hsTb, -i0, axis=1)) * 0.125),
                "res": np.ascontiguousarray(hsTb[:, i0 : i0 + SQ]) + bias,
                "wqT": wqT,
                "wkT": wkT,
                "wvT": wvT,
                "woT": woT,
            }
        )
    return in_maps


def assemble(results) -> np.ndarray:
    y = np.empty((B, S, C), np.float32)
    for c in range(NCORES):
        b, g = divmod(c, GROUP)
        i0 = g * SQ
        y[b, i0 : i0 + SQ, :] = np.asarray(results[c]["out"], np.float32).T
    return y


def kernel(**inputs) -> np.ndarray:
    from concourse.bass_utils import run_bass_kernel_spmd

    nc = get_nc()
    in_maps = make_in_maps(**inputs)
    res = run_bass_kernel_spmd(nc, in_maps, list(range(NCORES)))
    return assemble(res.results)


if __name__ == "__main__":
    import reference

    inputs = {k: np.asarray(v) for k, v in reference.setup_inputs().items()}
    got = kernel(**inputs)
    want = np.asarray(reference.reference(**inputs))
    err = np.linalg.norm(got - want) / np.linalg.norm(want)
    print("Relative error:", err)


# revision 49
# speedup vs baseline: 1.0144x; 1.0018x over previous
"""Multi-head attention (AttnProcessor2_0) on 8 TRN2 NeuronCores.

Problem: B=2, S=4096, C=640, H=10, Dh=64.
  q/k/v = hs @ W{q,k,v}.T ; per-head scores = q k^T / 8 ; softmax ;
  out = probs v ; y = out @ Wo.T + b_out + hs

Sharding (no collectives): core c -> batch b=c//4, query block g=c%4
(1024 queries).  Each core recomputes full K/V for its batch (head-dim
on partitions), computes its own S/4 x S attention block, output
projection, bias+residual.  Host passes hidden states TRANSPOSED and
ROLLED by the query offset so the same SPMD program works on every
core (softmax+PV are permutation-invariant along the key axis).

Device layout (feature-on-partition, token-on-free):
  kT [640, 4096] (5 chunks of 128 = 2 heads each)
  qT [5][128, 1024] pair layout: head 2i on partitions 0:64, head
     2i+1 on 64:128 (natural projection output, no zero padding)
  QK row-tiled pairs: two K=64 matmuls (row groups 0-1 / 2-3 via
     base-partition slicing) run CONCURRENTLY on the PE -> both heads'
     [128 keys x 512 q] score blocks in ~512 cycles instead of 1024.
  v  [4096, 650] (65-stride per head: 64 cols + ones col -> softmax
     denominators fall out of the PV matmul as PSUM row 64)
  probs: scoresT in PSUM -> ScalarE exp -> bf16 SBUF
  normalization: reciprocal_approx_fast of denom row, rank-1 PE outer
     product to broadcast across partitions, DVE mult into pair-packed
     attn tiles [128, 1024] (head 2i rows 0:64, head 2i+1 rows 64:128)
  oproj: pair-packed -> 5 matmuls of full K=128 contraction per
     (128-out-chunk, 512q), Wo tiles loaded as direct [128,128] DMAs.
Loop order: head-pair OUTER, query-tile inner -> background projection
windows are 2x wider; K/Q/V projections and oproj(it0) ride the PE
slack inside the attention loop (2 spare PSUM banks, tag "pp").
All matmuls bf16 (f32 PSUM accumulation).
"""

import sys

if "/opt/trn_rl_repo" not in sys.path:
    sys.path.insert(0, "/opt/trn_rl_repo")

from contextlib import ExitStack

import ml_dtypes
import numpy as np

import concourse.bass as bass
import concourse.tile as tile
from concourse import mybir
from concourse.bass import ts

BF16 = mybir.dt.bfloat16
F32 = mybir.dt.float32
F8 = mybir.dt.float8e4
DR = mybir.MatmulPerfMode.DoubleRow

B, S, C = 2, 4096, 640
H, DH = 10, 64
NCORES = 8
GROUP = 4  # cores per batch element
SQ = S // GROUP  # 1024 queries per core
SCALE = 0.125  # 1/sqrt(64)
CCH = C // 128  # 5 feature chunks (2 heads each)
NJT = S // 512  # 8 key tiles for K proj
NJC = S // 128  # 32 key chunks for attention
NIT = SQ // 512  # 2 query tiles
VST = DH + 1  # 65: per-head stride in v tiles (ones col appended)
NPAIR = H // 2  # 5 head pairs

# DVE-offloaded exp: fp8e4m3 bits of exp(SCALE*x) are approximately the
# affine x*A8+B8 (Schraudolph bit trick), so a single DVE tensor_scalar
# with int8 output replaces a ScalarE ACTIVATE for ~25% of the softmax --
# the two engines split the 41.9M-element exp stream.  The +-3% sawtooth
# is random across keys and self-normalizes through the denominators.
EXP_A8 = SCALE * 8.0 / np.log(2.0)
EXP_B8 = 55.54


def build_nc() -> bass.Bass:
    nc = bass.Bass()
    # hsT/W{q,k,v} travel as fp8e4 with hs pre-scaled by 1/8 and W by 8 on
    # the host (products exact): both land in fp8's normal range, and the
    # K/Q/V projections run DoubleRow (256-row virtual contraction, ~1.77x).
    hsT = nc.declare_dram_parameter("hsT", [C, S], F8, isOutput=False)
    res = nc.declare_dram_parameter("res", [C, SQ], F32, isOutput=False)
    wqT = nc.declare_dram_parameter("wqT", [C, C], F8, isOutput=False)
    wkT = nc.declare_dram_parameter("wkT", [C, C], F8, isOutput=False)
    wvT = nc.declare_dram_parameter("wvT", [C, C], F8, isOutput=False)
    woT = nc.declare_dram_parameter("woT", [C, C], BF16, isOutput=False)
    out = nc.declare_dram_parameter("out", [C, SQ], F32, isOutput=True)

    with ExitStack() as ctx:
        tc = ctx.enter_context(tile.TileContext(nc))
        # outer pool: tensors whose lifetime spans projections AND attention
        sb = ctx.enter_context(tc.tile_pool(name="sb", bufs=1))

        kT_sb = [sb.tile([128, S], BF16, tag=f"kT{i}", name=f"kT{i}") for i in range(CCH)]
        # per-head q, zero-padded to full 128-row contraction: the PE cost of
        # a matmul is its moving-column count, so padding the contraction
        # with zero rows is cycle-free and exact (row-tiled K=64 pairs would
        # halve QK, but this walrus build cannot codegen tile_position).
        qTz_sb = [
            [sb.tile([128, SQ], BF16, tag=f"qz{i}_{p}", name=f"qz{i}_{p}")
             for p in range(2)]
            for i in range(CCH)
        ]
        # v in DoubleRow layout [p, head, kc-pair, VSTP]: tile jc2 stacks key
        # chunks 2*jc2 / 2*jc2+1 in the pair dim, fp8 -> PV contracts 256
        # keys per matmul.  VSTP pads the per-head stride so the pair-dim
        # stride (VSTP bytes) satisfies the dual-fp8 LDW %16 rule.
        VSTP = 80
        v_dr = [sb.tile([128, H, 2, VSTP], F8, tag=f"v{j}", name=f"v{j}")
                for j in range(NJC // 2)]
        attn_sb = [sb.tile([128, SQ], BF16, tag=f"attn{p}", name=f"attn{p}")
                   for p in range(NPAIR)]
        ones_sb = sb.tile([128, DH], BF16, tag="ones", name="ones")
        nc.vector.memset(ones_sb[:], 1.0)
        ones32_sb = sb.tile([DH + 1, DH], F32, tag="ones32", name="ones32")
        nc.vector.memset(ones32_sb[:], 1.0)
        wo_sb = [sb.tile([128, C], BF16, tag=f"wo{p}", name=f"wo{p}")
                 for p in range(NPAIR)]
        # output-projection accumulator: partial sums over head pairs land
        # here as each pair's normalization completes, so the epilogue only
        # waits on the LAST pair instead of running 50 matmuls cold.
        oacc_sb = [sb.tile([128, SQ], F32, tag=f"oacc{e}", name=f"oacc{e}")
                   for e in range(CCH)]

        # ---------------- load + first projections ----------------
        # Minimal critical path to the first exp: wk/wq column-slices for
        # feature chunk dc0 + hsT tokens 0:1024, then kproj(0,jt0)/qproj(0).
        load = ctx.enter_context(tc.tile_pool(name="load", bufs=1))
        # whole-tensor tiles [128, chunk(5), free]: feature chunk c of the
        # [C, *] DRAM tensor lands at [:, c, :] via one rearranged-AP DMA
        # (DMA dispatch costs ~0.6us per instruction on an engine queue, so
        # fewer+larger transfers win the startup race).  DoubleRow pair j
        # is the [:, 2j:2j+2, :] slice; the 5th chunk rides plain fp8.
        hs_sb = load.tile([128, CCH, S], F8, tag="hs", name="hs")
        nc.sync.dma_start(
            hs_sb[:, :, 0:SQ],
            hsT[:, 0:SQ].rearrange("(c p) t -> p c t", c=CCH),
        )

        def dma_w(name, src, eng):
            w = load.tile([128, CCH, C], F8, tag=name, name=name)
            eng.dma_start(w[:, :, :], src[:, :].rearrange("(c p) m -> p c m", c=CCH))
            return w

        wk_sb = dma_w("wk", wkT, nc.sync)
        wq_sb = dma_w("wq", wqT, nc.sync)
        wv_sb = dma_w("wv", wvT, nc.sync)

        def emit_hsT_tail():
            # deferred until after the first exp so ScalarE's conservative
            # vector-clock waits don't cover this 2MB of DMA
            nc.sync.dma_start(
                hs_sb[:, :, SQ:S],
                hsT[:, SQ:S].rearrange("(c p) t -> p c t", c=CCH),
            )

        def emit_wo_dma():
            for p in range(NPAIR):
                nc.gpsimd.dma_start(wo_sb[p][:], woT[ts(p, 128), :])

        def emit_kproj(dc, jt, pool):
            ps = pool.tile([128, 512], F32, tag="pp", name="pp", bufs=2)
            for j in range(2):
                nc.tensor.matmul(
                    ps[:],
                    wk_sb[:, 2 * j : 2 * j + 2, ts(dc, 128)],
                    hs_sb[:, 2 * j : 2 * j + 2, ts(jt, 512)],
                    start=(j == 0),
                    stop=False,
                    perf_mode=DR,
                )
            nc.tensor.matmul(
                ps[:],
                wk_sb[:, 4, ts(dc, 128)],
                hs_sb[:, 4, ts(jt, 512)],
                start=False,
                stop=True,
            )
            nc.vector.tensor_copy(kT_sb[dc][:, ts(jt, 512)], ps[:])

        def emit_qproj(dc, it, pool):
            ps = pool.tile([128, 512], F32, tag="pp", name="pp", bufs=2)
            for j in range(2):
                nc.tensor.matmul(
                    ps[:],
                    wq_sb[:, 2 * j : 2 * j + 2, ts(dc, 128)],
                    hs_sb[:, 2 * j : 2 * j + 2, ts(it, 512)],
                    start=(j == 0),
                    stop=False,
                    perf_mode=DR,
                )
            nc.tensor.matmul(
                ps[:],
                wq_sb[:, 4, ts(dc, 128)],
                hs_sb[:, 4, ts(it, 512)],
                start=False,
                stop=True,
            )
            nc.vector.tensor_copy(qTz_sb[dc][0][0:DH, ts(it, 512)], ps[0:DH, :])
            nc.vector.tensor_copy(qTz_sb[dc][1][DH:128, ts(it, 512)], ps[DH:128, :])

        def emit_vproj(jc, pool):
            v3 = v_dr[jc // 2][:, :, jc % 2, :]  # [128, H, VSTP]
            nc.vector.memset(v3[:, :, DH : DH + 1], 1.0)
            for d0, dn in ((0, 512), (512, 128)):
                ps = pool.tile([128, 512], F32, tag="pp", name="pp", bufs=2)
                for j in range(2):
                    nc.tensor.matmul(
                        ps[:, 0:dn],
                        hs_sb[:, 2 * j : 2 * j + 2, ts(jc, 128)],
                        wv_sb[:, 2 * j : 2 * j + 2, d0 : d0 + dn],
                        start=(j == 0),
                        stop=False,
                        perf_mode=DR,
                    )
                nc.tensor.matmul(
                    ps[:, 0:dn],
                    hs_sb[:, 4, ts(jc, 128)],
                    wv_sb[:, 4, d0 : d0 + dn],
                    start=False,
                    stop=True,
                )
                nc.vector.tensor_copy(
                    v3[:, d0 // DH : (d0 + dn) // DH, 0:DH],
                    ps[:, 0:dn].rearrange("p (h x) -> p h x", x=DH),
                )

        with tc.tile_pool(name="pp0", bufs=2, space="PSUM") as pp0:
            for dc in range(CCH):
                nc.vector.memset(qTz_sb[dc][0][DH:128, :], 0.0)
                nc.vector.memset(qTz_sb[dc][1][0:DH, :], 0.0)
            emit_kproj(0, 0, pp0)
            for it in range(NIT):
                emit_qproj(0, it, pp0)
            emit_kproj(0, 1, pp0)

        # ---------------- attention phase ----------------
        with tc.tile_pool(name="ap", bufs=1, space="PSUM") as ap, \
             tc.tile_pool(name="pt", bufs=4) as pt_pool, \
             tc.tile_pool(name="ob", bufs=3) as ob, \
             tc.tile_pool(name="scratch", bufs=1) as scratch:

            def norm_stage1(hp, pvA, pvB, it):
                # Fast, frees the pv banks: copy raw pv rows + denom row to
                # SBUF, then DMA-reshape both denom rows [1,512] into a
                # [128,8] tile so ONE cheap 8-element-per-lane reciprocal
                # serves both heads (a [1,512] reciprocal costs 3.3us of
                # head-of-line DVE queue time; [128,8] costs ~0.1us).
                raws, dens = [], []
                for half, pv in ((0, pvA), (1, pvB)):
                    raw = scratch.tile([DH, 512], BF16, tag=f"raw{half}",
                                       name=f"raw{half}")
                    nc.vector.tensor_copy(raw[:], pv[0:DH, :])
                    den = scratch.tile([DH + 1, 512], F32, tag=f"den{half}",
                                       name=f"den{half}")
                    nc.vector.tensor_copy(den[DH : DH + 1, :],
                                          pv[DH : DH + 1, :])
                    raws.append(raw)
                    dens.append(den)
                den2 = scratch.tile([128, 8], F32, tag="den2", name="den2")
                for half in range(2):
                    nc.sync.dma_start(
                        den2[:, half * 4 : half * 4 + 4],
                        dens[half][DH : DH + 1, :],
                    )
                return (hp, it, raws, den2)

            def norm_stage2(state):
                # One batched reciprocal, cast to bf16 (tiny), DMA rows back.
                hp, it, raws, den2 = state
                rcp2 = scratch.tile([128, 8], F32, tag="rcp2", name="rcp2")
                nc.vector.reciprocal(rcp2[:], den2[:])
                rcp2b = scratch.tile([128, 8], BF16, tag="rcp2b", name="rcp2b")
                with nc.allow_low_precision(reason="softmax recip bf16"):
                    nc.vector.tensor_copy(rcp2b[:], rcp2[:])
                rcps = []
                for half in range(2):
                    rcp = scratch.tile([DH + 1, 512], BF16, tag=f"rcp{half}",
                                       name=f"rcp{half}")
                    nc.sync.dma_start(
                        rcp[DH : DH + 1, :],
                        rcp2b[:, half * 4 : half * 4 + 4],
                    )
                    rcps.append(rcp)
                return (hp, it, raws, rcps)

            def norm_stage3(state):
                # rank-1 PE broadcast of 1/den across 64 partitions (DVE
                # cannot cross partitions), then DVE mult into attn pair.
                hp, it, raws, rcps = state
                isl = ts(it, 512)
                for half in range(2):
                    bc = ap.tile([128, 512], F32, tag="pp", name="pp", bufs=2)
                    nc.tensor.matmul(
                        bc[0:DH, :],
                        ones_sb[DH : DH + 1, :],
                        rcps[half][DH : DH + 1, :],
                        start=True,
                        stop=True,
                    )
                    nc.vector.tensor_mul(
                        attn_sb[hp][half * DH : (half + 1) * DH, isl],
                        raws[half][:], bc[0:DH, :],
                    )

            def emit_opart(hp, it, ec):
                # one head pair's contribution to output chunk (ec, it),
                # accumulated into SBUF right after the pair's norm lands
                isl = ts(it, 512)
                ps = ap.tile([128, 512], F32, tag="pp", name="pp", bufs=2)
                nc.tensor.matmul(
                    ps[:],
                    wo_sb[hp][:, ts(ec, 128)],
                    attn_sb[hp][:, isl],
                    start=True,
                    stop=True,
                )
                if hp == 0:
                    nc.vector.tensor_copy(oacc_sb[ec][:, isl], ps[:])
                else:
                    nc.vector.tensor_add(oacc_sb[ec][:, isl],
                                         oacc_sb[ec][:, isl], ps[:])

            def emit_ofinal(it, ec):
                isl = ts(it, 512)
                rt = ob.tile([128, 512], F32, tag="rt", name="rt", bufs=2)
                nc.sync.dma_start(rt[:], res[ts(ec, 128), isl])
                ot = ob.tile([128, 512], F32, tag="ot", name="ot", bufs=2)
                nc.vector.tensor_add(ot[:], oacc_sb[ec][:, isl], rt[:])
                nc.sync.dma_start(out[ts(ec, 128), isl], ot[:])

            # background work queue: one item is popped per (kc) step.
            bg = []
            bg.extend((lambda jt=jt: emit_kproj(0, jt, ap)) for jt in range(2, NJT))

            pend_norm = None
            for hp in range(NPAIR):
                if hp + 1 < NPAIR:
                    # next pair's K/Q projections ride this pair's window
                    bg.extend(
                        (lambda jt=jt, dc=hp + 1: emit_kproj(dc, jt, ap))
                        for jt in range(NJT)
                    )
                    bg.extend(
                        (lambda q_it=q_it, dc=hp + 1: emit_qproj(dc, q_it, ap))
                        for q_it in range(NIT)
                    )
                for it in range(NIT):
                    isl = ts(it, 512)
                    pvA = ap.tile([DH + 1, 512], F32, tag="pvA", bufs=1, name="pvA")
                    pvB = ap.tile([DH + 1, 512], F32, tag="pvB", bufs=1, name="pvB")
                    prev, pend_norm = pend_norm, None
                    nst = {}

                    def emit_qk(jc):
                        sc = ap.tile([128, 1024], F32, tag="sc", bufs=2,
                                     name="sc")
                        nc.tensor.matmul(
                            sc[:, 0:512],
                            kT_sb[hp][:, ts(jc, 128)],
                            qTz_sb[hp][0][:, isl],
                            start=True,
                            stop=True,
                        )
                        nc.tensor.matmul(
                            sc[:, 512:1024],
                            kT_sb[hp][:, ts(jc, 128)],
                            qTz_sb[hp][1][:, isl],
                            start=True,
                            stop=True,
                        )
                        return sc

                    # QK runs one step ahead of the exp stream: the next
                    # scores are issued to the PE right after this step's
                    # exp is enqueued, so ScalarE never waits on a QK stuck
                    # behind PV/background work in the PE queue.
                    sc_next = emit_qk(0)
                    for jc in range(NJC):
                        sc = sc_next
                        if jc % 2 == 0:
                            # [p, head(2), kc-pair(2), q]: fp8 probs laid out
                            # so PV can contract 256 keys via DoubleRow
                            pt = pt_pool.tile([128, 2, 2, 512], F8, tag="pt",
                                              name="pt")
                        if not (hp == 0 and it == 0) and jc % 4 == 1:
                            nc.vector.tensor_scalar(
                                pt[:, :, jc % 2, :].bitcast(mybir.dt.int8),
                                sc[:].rearrange("p (h q) -> p h q", q=512),
                                EXP_A8,
                                EXP_B8,
                                mybir.AluOpType.mult,
                                mybir.AluOpType.add,
                            )
                        else:
                            nc.scalar.activation(
                                pt[:, :, jc % 2, :],
                                sc[:].rearrange("p (h q) -> p h q", q=512),
                                mybir.ActivationFunctionType.Exp,
                                bias=0.0, scale=SCALE,
                            )
                        if jc + 1 < NJC:
                            sc_next = emit_qk(jc + 1)
                        # staged normalization + output-projection of the
                        # PREVIOUS (pair, it): stages are spaced so DMA
                        # round-trips complete before their consumer enters
                        # the strict-FIFO DVE queue (no head-of-line
                        # blocking of the projection PSUM-evacuation
                        # copies), and the opart/ofinal work dribbles out
                        # one chunk per step.
                        if prev is not None:
                            phr, _, _, pit = prev
                            if jc == 0:
                                nst[1] = norm_stage1(*prev)
                            elif jc == 6:
                                nst[2] = norm_stage2(nst[1])
                            elif jc == 12:
                                norm_stage3(nst[2])
                            elif 13 <= jc < 13 + CCH:
                                emit_opart(phr, pit, jc - 13)
                            elif (phr == NPAIR - 1
                                  and 18 <= jc < 18 + CCH):
                                emit_ofinal(pit, jc - 18)
                        if hp == 0 and it == 0:
                            if jc == 0:
                                emit_hsT_tail()
                            if jc == 1:
                                emit_wo_dma()
                            # V projection rides inside the first pair's
                            # window, each chunk just ahead of its PV
                            emit_vproj(jc, ap)
                            if jc % 3 == 2 and bg:
                                bg.pop(0)()
                        elif hp == 0 and it == 1:
                            # drain remaining kproj/qproj for the next pair
                            if bg:
                                bg.pop(0)()
                            if jc % 2 == 0 and bg:
                                bg.pop(0)()
                        elif bg:
                            bg.pop(0)()
                        if jc % 2 == 1:
                            jc2 = jc // 2
                            for half, pv in ((0, pvA), (1, pvB)):
                                h = 2 * hp + half
                                nc.tensor.matmul(
                                    pv[:],
                                    v_dr[jc2][:, h, :, 0:VST],
                                    pt[:, half, :, :],
                                    start=(jc2 == 0),
                                    stop=(jc2 == NJC // 2 - 1),
                                    perf_mode=DR,
                                )
                    pend_norm = (hp, pvA, pvB, it)
                # barrier: next pair's kT/qT must be fully emitted before
                # its first QK reads them
                while bg:
                    bg.pop(0)()
            s1 = norm_stage1(*pend_norm)
            s2 = norm_stage2(s1)
            norm_stage3(s2)
            for ec in range(CCH):
                emit_opart(NPAIR - 1, 1, ec)
            for ec in range(CCH):
                emit_ofinal(1, ec)

    _dedupe_ldweights(nc)
    _spill_matmul_waits(nc)
    return nc


def _dedupe_ldweights(nc: bass.Bass) -> None:
    """Drop a LDWEIGHTS that reloads the stationary operand the PE already
    holds (e.g. the two QK matmuls of a head pair share one kT chunk).
    Matmuls do not invalidate loaded weights; any other PE-engine
    instruction conservatively resets the tracked state."""
    for f in nc.m.functions:
        for blk in f.blocks:
            keep = []
            prev_sig = None
            mm_engine = None
            for inst in blk.instructions:
                tn = type(inst).__name__
                if tn == "InstMatmult":
                    mm_engine = inst.engine
                    break
            for inst in blk.instructions:
                tn = type(inst).__name__
                if tn == "InstLdweights":
                    si = inst.sync_info
                    clean = si is None or (not si.on_wait and not si.on_update)
                    sig = repr(inst.ins[0]) + repr(getattr(inst, "perf_mode", None))
                    if clean and sig == prev_sig:
                        continue
                    prev_sig = sig
                elif tn in ("InstMatmult", "InstEventSemaphore"):
                    pass
                elif getattr(inst, "engine", None) == mm_engine:
                    prev_sig = None
                keep.append(inst)
            blk.instructions[:] = keep


# walrus embedded-sync-wait capacity per BIR opcode.  Matmult holds a
# single wait; excess waits hoist onto the paired Ldweights (in-order
# issue on PE makes that equivalent).  Other compute ops spill onto
# EventSemaphore carrier instructions inserted just before them on the
# same engine.  DMACopy / Drain / EventSemaphore handle many waits
# natively (bacc emits such itself) and are left alone.
_WAIT_CAPS = {
    "InstMatmult": 1,
    "InstLdweights": 1,
    "InstActivation": 1,
    "InstReciprocal": 1,
    "InstTensorTensor": 1,
    "InstTensorCopy": 1,
    "InstTensorScalarPtr": 1,
    "InstTensorReduce": 1,
    "InstMemset": 1,
    "InstDMACopy": 1,
    "InstDrain": 1,
    "InstCustomDveAnt": 1,
}
_ES_CAP = 2  # waits per EventSemaphore carrier (walrus: <=2 waits, <=1 update)


def _spill_matmul_waits(nc: bass.Bass) -> None:
    spill_id = [0]

    def carriers(excess, engine):
        out = []
        for i in range(0, len(excess), _ES_CAP):
            es = mybir.InstEventSemaphore(
                name=f"wait-spill-{spill_id[0]}", ins=[], outs=[]
            )
            spill_id[0] += 1
            es.engine = engine
            es.sync_info = mybir.SyncInfo(
                on_wait=excess[i : i + _ES_CAP], on_update=[]
            )
            out.append(es)
        return out

    for f in nc.m.functions:
        for blk in f.blocks:
            insts = blk.instructions
            i = 0
            while i < len(insts):
                inst = insts[i]
                tn = type(inst).__name__
                cap = _WAIT_CAPS.get(tn)
                si = inst.sync_info
                if cap is None or si is None or len(si.on_wait) <= cap:
                    i += 1
                    continue
                w = list(si.on_wait)
                if tn == "InstMatmult" and cap == 1:
                    # Keep the latest-satisfied dependency (the ACT-produced
                    # operand, e.g. probs from exp) embedded on the matmul and
                    # hoist early ones onto the Ldweights: a wait on the LDW
                    # blocks its background prefetch and serializes ~50ns of
                    # weight-load into every PV matmul.
                    acts = [x for x in w if "Activation" in (x.ant_name or "")]
                    if acts:
                        keep = [acts[-1]]
                        excess = [x for x in w if x is not acts[-1]]
                    else:
                        keep, excess = w[-cap:], w[:-cap]
                else:
                    keep, excess = w[-cap:], w[:-cap]
                prev = insts[i - 1] if i > 0 else None
                if (
                    tn == "InstMatmult"
                    and prev is not None
                    and type(prev).__name__ == "InstLdweights"
                    and len(((prev.sync_info and prev.sync_info.on_wait) or []))
                    + len(excess) <= 1
                ):
                    psi = prev.sync_info
                    pw = list(psi.on_wait) if psi is not None else []
                    pu = list(psi.on_update) if psi is not None else []
                    prev.sync_info = mybir.SyncInfo(on_wait=pw + excess, on_update=pu)
                else:
                    new = carriers(excess, inst.engine)
                    insts[i:i] = new
                    i += len(new)
                inst.sync_info = mybir.SyncInfo(
                    on_wait=keep, on_update=list(si.on_update)
                )
                i += 1


_CACHED_NC = None


def get_nc() -> bass.Bass:
    global _CACHED_NC
    if _CACHED_NC is None:
        _CACHED_NC = build_nc()
    return _CACHED_NC


def make_in_maps(hidden_states, Wq, Wk, Wv, Wo, b_out):
    hs = np.asarray(hidden_states, dtype=np.float32)
    bf = ml_dtypes.bfloat16
    f8 = ml_dtypes.float8_e4m3

    def to_f8(x):
        return np.clip(x, -224.0, 224.0).astype(f8)

    # hs scaled by 1/8 and W by 8 (exact products) so both sit in fp8e4's
    # normal range (w_std=0.02 would otherwise be mostly denormal).
    wqT = to_f8(np.ascontiguousarray(np.asarray(Wq, np.float32).T) * 8.0)
    wkT = to_f8(np.ascontiguousarray(np.asarray(Wk, np.float32).T) * 8.0)
    wvT = to_f8(np.ascontiguousarray(np.asarray(Wv, np.float32).T) * 8.0)
    woT = np.ascontiguousarray(np.asarray(Wo, np.float32).T).astype(bf)
    bias = np.asarray(b_out, np.float32).reshape(C, 1)
    in_maps = []
    for c in range(NCORES):
        b, g = divmod(c, GROUP)
        i0 = g * SQ
        hsTb = hs[b].T  # [C, S]
        in_maps.append(
            {
                "hsT": to_f8(np.ascontiguousarray(np.roll(hsTb, -i0, axis=1)) * 0.125),
                "res": np.ascontiguousarray(hsTb[:, i0 : i0 + SQ]) + bias,
                "wqT": wqT,
                "wkT": wkT,
                "wvT": wvT,
                "woT": woT,
            }
        )
    return in_maps


def assemble(results) -> np.ndarray:
    y = np.empty((B, S, C), np.float32)
    for c in range(NCORES):
        b, g = divmod(c, GROUP)
        i0 = g * SQ
        y[b, i0 : i0 + SQ, :] = np.asarray(results[c]["out"], np.float32).T
    return y


def kernel(**inputs) -> np.ndarray:
    from concourse.bass_utils import run_bass_kernel_spmd

    nc = get_nc()
    in_maps = make_in_maps(**inputs)
    res = run_bass_kernel_spmd(nc, in_maps, list(range(NCORES)))
    return assemble(res.results)


if __name__ == "__main__":
    import reference

    inputs = {k: np.asarray(v) for k, v in reference.setup_inputs().items()}
    got = kernel(**inputs)
    want = np.asarray(reference.reference(**inputs))
    err = np.linalg.norm(got - want) / np.linalg.norm(want)
    print("Relative error:", err)
